# revision 17
# baseline (speedup 1.0000x reference)
"""DiffNet social GCN (2-hop) on 8 TRN2 NeuronCores.

Reference computation (all f32):
    x = user_embs                                  # [N, H]
    for k in range(2):
        agg = adj @ x                              # [N, H]
        x = tanh(concat([agg, x], 1) @ W[k])       # [N, H]

Distribution: row-shard adj across 8 cores (each core owns N/8 = 2048
destination rows). The 1 GiB adj matrix is streamed from HBM once per
hop per core; an 8-core AllGather shares the hop-1 activations.

Device-side layout choices (all prepared on the host in kernel()):
  * adjt  [N, 2048]  = adj[rows_i, :].T  — contiguous, so aggregation
    matmuls get their contraction dim (adj columns) on SBUF partitions
    with plain contiguous DMAs and zero on-chip transposes.
  * x0r   [128, 8, 1024] — user_embs in "chunk-major" layout:
    x0r[p, i, 64*c' + h] = x[128*(16i+c') + p, h]; a k-chunk's 128x64
    stationary operand is a plain slice. The same layout is exactly what
    an AllGather of per-core [128, 1024] shards produces, so hop 2 reads
    the gathered activations with one contiguous DMA.
  * wpack [64, 256] = per-hop W split into the agg-half and x-half.

The aggregation computes aggT = (adj_i @ x)^T = sum_k x_k^T-style PE
matmuls (stationary = x chunk [128,64], moving = adjt tile [128,512]),
accumulated f32 in PSUM. The dense stage then computes
hT = tanh(Wa^T @ aggT + Wb^T @ xT_own) directly in the transposed
layout, and 16 PE transposes bring each hop's activations back to the
natural layout for the AllGather / final output.
"""

import numpy as np

N = 16384
H = 64
P = 128
NCORES = 8
ROWS = N // NCORES            # 2048 destination rows per core
CHUNKS = N // P               # 128 contraction chunks
CH_OWN = ROWS // P            # 16 chunks owned per core
SLAB_CH = 4                   # k-chunks per adj DMA (4 MiB per transfer)
NSLABS = CHUNKS // SLAB_CH    # 32
NTILE = 512                   # fp32 moving-operand free dim
NT = ROWS // NTILE            # 4

_CACHE = {}
LAST_RESULT = None


def _build_nc():
    import concourse.bass as bass
    import concourse.mybir as mybir
    import concourse.tile as tile
    from concourse import bacc
    from concourse.masks import make_identity

    f32 = mybir.dt.float32
    # Bacc (not plain Bass): its compile() runs generate_event_semaphores(),
    # which legalizes multi-semaphore waits into InstEventSemaphore — walrus
    # allows at most one sync wait per regular instruction.
    nc = bacc.Bacc(num_devices=NCORES)

    adjt = nc.declare_dram_parameter("adjt", [N, ROWS], f32, isOutput=False)
    x0r = nc.declare_dram_parameter("x0r", [P, NCORES, CH_OWN * H], f32, isOutput=False)
    x0t = nc.declare_dram_parameter("x0t", [H, ROWS], f32, isOutput=False)
    wpack = nc.declare_dram_parameter("wpack", [H, 4 * H], f32, isOutput=False)
    out = nc.declare_dram_parameter("out", [P, CH_OWN * H], f32, isOutput=True)

    # [p, c, r]: k-chunk c, node-within-chunk p, destination row r
    adjt_r = adjt.ap().rearrange("(c p) r -> p c r", p=P)

    def ds(start, size):
        return bass.ds(start, size)

    with tile.TileContext(nc) as tc:
        with (
            tc.tile_pool(name="slab", bufs=2) as slab_pool,
            tc.tile_pool(name="xr", bufs=2) as xr_pool,
            tc.tile_pool(name="small", bufs=1) as small_pool,
            tc.tile_pool(name="act", bufs=1) as act_pool,
            tc.tile_pool(name="psA", bufs=1, space="PSUM") as psA,
            tc.tile_pool(name="psD", bufs=1, space="PSUM") as psD,
            tc.tile_pool(name="psT", bufs=1, space="PSUM") as psT,
            tc.tile_pool(name="dram", bufs=1, space="DRAM") as dram_pool,
        ):
            # ident also doubles as the rhs for "wait absorber" transposes:
            # every PE instruction may carry at most ONE semaphore wait in the
            # lowered LDWEIGHTS slot, so each DMA-produced tile is first
            # touched by a throwaway PE transpose (1 wait each) before the
            # real matmuls consume it.
            ident = small_pool.tile([P, P], f32, name="ident")
            make_identity(nc, ident)

            # One persistent scratch tile for all absorber writes: same-tile
            # WAW on the same engine is elided, so each absorber carries only
            # its DMA wait (never a slot-release wait).
            dummy = psD.tile([P, P], f32, name="dummy")

            def absorb(in_ap, idn):
                # throwaway PE transpose whose only job is to carry the
                # single semaphore wait for `in_ap`'s producer
                nc.tensor.transpose(
                    dummy[0 : in_ap.shape[-1], 0 : in_ap.shape[0]], in_ap, idn
                )

            absorb(ident[0:P, 0:P], ident[0:P, 0:P])

            w_sb = small_pool.tile([H, 4 * H], f32, name="w_sb")
            nc.sync.dma_start(w_sb[:], wpack.ap())
            absorb(w_sb[:, 0:P], ident[0:H, 0:H])

            x0t_sb = small_pool.tile([H, ROWS], f32, name="x0t_sb")
            nc.sync.dma_start(x0t_sb[:], x0t.ap())
            absorb(x0t_sb[:, 0:P], ident[0:H, 0:H])

            ag_in = dram_pool.tile([P, CH_OWN * H], f32, name="ag_in")
            ag_out = dram_pool.tile(
                [NCORES * P, CH_OWN * H], f32, name="ag_out", addr_space="Shared"
            )

            xs0 = xr_pool.tile([P, NCORES, CH_OWN * H], f32, tag="xr", name="xs0")
            nc.sync.dma_start(xs0[:], x0r.ap())
            absorb(xs0[:, 0, 0:H], ident[0:P, 0:P])

            xs = xs0
            xt = x0t_sb
            for hop in range(2):
                aggT = psA.tile([H, ROWS], f32, tag="aggT", name=f"aggT{hop}")
                for j in range(NSLABS):
                    slab = slab_pool.tile(
                        [P, SLAB_CH, ROWS], f32, tag="slab", name=f"slab{hop}_{j}"
                    )
                    nc.sync.dma_start(
                        slab[:], adjt_r[:, j * SLAB_CH : (j + 1) * SLAB_CH, :]
                    )
                    for s in range(SLAB_CH):
                        k = j * SLAB_CH + s
                        lhsT = xs[:, k // CH_OWN, ds(H * (k % CH_OWN), H)]
                        for n in range(NT):
                            nc.tensor.matmul(
                                aggT[:, ds(n * NTILE, NTILE)],
                                lhsT,
                                slab[:, s, ds(n * NTILE, NTILE)],
                                start=(k == 0),
                                stop=(k == CHUNKS - 1),
                            )

                # dense + tanh, in transposed [H, ROWS] layout; the dense
                # matmuls reuse aggT's PSUM banks (its value was copied to
                # SBUF just before, region by region)
                wa = w_sb[:, ds(H * (2 * hop + 0), H)]
                wb = w_sb[:, ds(H * (2 * hop + 1), H)]
                ht_sb = act_pool.tile([H, ROWS], f32, tag="ht", bufs=2, name=f"ht{hop}")
                hraw = act_pool.tile([H, ROWS], f32, tag="hraw", bufs=2, name=f"hraw{hop}")
                for n in range(NT):
                    sl = ds(n * NTILE, NTILE)
                    aggT_sb = act_pool.tile(
                        [H, NTILE], f32, tag="aggsb", bufs=2, name=f"aggsb{hop}_{n}"
                    )
                    nc.vector.tensor_copy(aggT_sb[:], aggT[:, sl])
                    nc.tensor.matmul(
                        aggT[:, sl], wa, aggT_sb[:], start=True, stop=False
                    )
                    nc.tensor.matmul(aggT[:, sl], wb, xt[:, sl], start=False, stop=True)
                    # PSUM -> SBUF on DVE, then tanh reads SBUF: keeps ACT off
                    # PSUM so dense matmuls never pick up an ACT-hazard wait
                    nc.vector.tensor_copy(hraw[:, sl], aggT[:, sl])
                    nc.scalar.activation(
                        ht_sb[:, sl], hraw[:, sl], mybir.ActivationFunctionType.Tanh
                    )

                # back to natural layout: 16 PE transposes of [64,128] -> [128,64].
                # Two 1-bank PSUM tiles hold 8 transpose outputs each as
                # disjoint regions — no slot cycling, so every transpose
                # carries at most the one ACT (tanh) wait.
                xout = act_pool.tile(
                    [P, CH_OWN * H], f32, tag="xout", bufs=2, name=f"xout{hop}"
                )
                tpA = psT.tile([P, 8 * H], f32, tag="tpA", name=f"tpA{hop}")
                tpB = psT.tile([P, 8 * H], f32, tag="tpB", name=f"tpB{hop}")
                for c in range(CH_OWN):
                    tp = (tpA if c < 8 else tpB)[:, ds((c % 8) * H, H)]
                    nc.tensor.transpose(tp, ht_sb[:, ds(c * P, P)], ident[0:H, 0:H])
                    # ACT (not DVE) drains the transpose banks: the next
                    # transpose's bank-hazard wait then shares the Activation
                    # semaphore with its tanh dependency (1 wait per matmul)
                    nc.scalar.copy(xout[:, ds(c * H, H)], tp)

                if hop == 0:
                    # absorb the last hraw DVE-copy tick onto PE so hop 1's
                    # first matmul doesn't carry aggT's slot-release wait
                    absorb(hraw[:, ds(3 * NTILE, P)], ident[0:H, 0:H])
                    nc.sync.dma_start(ag_in[:], xout[:])
                    nc.gpsimd.collective_compute(
                        "AllGather",
                        mybir.AluOpType.bypass,
                        replica_groups=[list(range(NCORES))],
                        ins=[ag_in[:].opt()],
                        outs=[ag_out[:].opt()],
                    )
                    xs1 = xr_pool.tile(
                        [P, NCORES, CH_OWN * H], f32, tag="xr", name="xs1"
                    )
                    nc.sync.dma_start(
                        xs1[:], ag_out[:].rearrange("(i p) f -> p i f", p=P)
                    )
                    absorb(xs1[:, 0, 0:H], ident[0:P, 0:P])
                    xs = xs1
                    xt = ht_sb
                else:
                    nc.sync.dma_start(out.ap(), xout[:])

    nc.finalize()
    return nc


def _get_nc():
    if "nc" not in _CACHE:
        _CACHE["nc"] = _build_nc()
    return _CACHE["nc"]


def _prepare_in_maps(user_embs, adj, W):
    ue = np.ascontiguousarray(user_embs, dtype=np.float32)
    adj = np.asarray(adj, dtype=np.float32)
    W = np.asarray(W, dtype=np.float32)

    # x0r[p, i, 64c' + h] = ue[128*(16i+c') + p, h]
    x0r = np.ascontiguousarray(
        ue.reshape(CHUNKS, P, H).transpose(1, 0, 2).reshape(P, NCORES, CH_OWN * H)
    )
    # wpack[:, 64*(2k+a) : ...] = W[k] rows [64a:64a+64]
    wpack = np.ascontiguousarray(
        np.concatenate([W[0][:H], W[0][H:], W[1][:H], W[1][H:]], axis=1)
    )

    in_maps = []
    for i in range(NCORES):
        rows = slice(i * ROWS, (i + 1) * ROWS)
        in_maps.append(
            {
                "adjt": np.ascontiguousarray(adj[rows, :].T),
                "x0r": x0r,
                "x0t": np.ascontiguousarray(ue[rows, :].T),
                "wpack": wpack,
            }
        )
    return in_maps


def _unshard(results):
    # out[p, 64c' + h] = x2[128c' + p, h] for the core's own rows
    shards = []
    for i in range(NCORES):
        o = results[i]["out"]
        shards.append(o.reshape(P, CH_OWN, H).transpose(1, 0, 2).reshape(ROWS, H))
    return np.ascontiguousarray(np.concatenate(shards, axis=0))


def kernel(user_embs: np.ndarray, adj: np.ndarray, W: np.ndarray) -> np.ndarray:
    global LAST_RESULT
    import os

    try:
        import antenv.axon_hooks  # noqa: F401
    except ImportError:
        # BASS_TRACE's axon NTFF path needs antenv.axon_hooks; fall back to
        # the plain execute path when the hook module isn't shipped.
        os.environ["BASS_NEVER_TRACE"] = "1"
    from concourse.bass_utils import run_bass_kernel_spmd

    in_maps = _prepare_in_maps(user_embs, adj, W)
    nc = _get_nc()
    LAST_RESULT = run_bass_kernel_spmd(nc, in_maps, list(range(NCORES)))
    return _unshard(LAST_RESULT.results)
